# revision 2
# baseline (speedup 1.0000x reference)
"""AttentionPool (segment softmax + weighted segment sum) on 8 TRN2 cores, v2.

kernel(x, batch, W1, b1, W2, b2) -> pooled [4096, 256] f32

Differences vs the v1 baseline (which was PE/ACT-bound on 8 PE-transposes +
PSUM->SBUF copies per 512-node block):
  - x is shipped in BOTH layouts from the host: node-on-partition blocks
    (`xd`, with a 257th ones-column for the softmax denominator) for the
    pooling matmul, and channel-on-partition blocks (`xt`) for the MLP.
    No on-device transposes, no PSUM->SBUF copies (XT_MODE="pe" restores the
    on-device transpose variant for comparison).
  - Pooling uses the one-hot weights oe as the *stationary* operand
    (32-column LDWEIGHTS) and x as the moving operand; output accumulates
    as [gw, 257] (graph-on-partition) in PSUM across the window's blocks,
    col 256 = denominator. Finalize is 3 DVE ops + DMA per window; the v1
    final PE transposes are gone.

Per 512-node block the PE does: 2 MLP matmuls (512 mov each), 4 score
matmuls (1 mov), 4 pooling matmuls (257 mov) -> ~2.06k moving columns.
"""

from contextlib import ExitStack

import numpy as np
import ml_dtypes

import concourse.bass as bass
import concourse.mybir as mybir
import concourse.tile as tile
from concourse import bacc, bass_utils
from concourse.masks import make_identity

FP32 = mybir.dt.float32
BF16 = mybir.dt.bfloat16
FP8 = mybir.dt.float8e4
C = 256
BLK = 512
KCH = BLK // 128
N_CORES = 8
GW = 32  # graphs per window
NWIN = 16  # windows per core -> 512 graphs per core

XT_MODE = "dma"  # "dma": host ships x^T; "pe": transpose on tensor engine
XT_DT = FP8  # dtype of the score-path x^T (BF16 or FP8 = float8e3, 4 mantissa bits)
# Pre-scales shift x and W1 into e3m4's normal range (min normal 0.25, max
# 15.5) so PE subnormal flushing costs nothing; descaled in tanh's scale.
X_PRESCALE = 2.8
W1_PRESCALE = 1.0
W1_DT = BF16  # stationary may stay bf16 while the moving operand is fp8

_NC_CACHE = {}


def _build_nc(nwin, nw, gw, repeat=1):
    # repeat > 1 re-runs the whole computation (same inputs/outputs) inside
    # one NEFF; used only for overhead-cancelling timing measurements.
    assert nw % BLK == 0 and gw <= 128
    bpw = nw // BLK
    nblk = nwin * bpw

    nc = bacc.Bacc(None, target_bir_lowering=False)

    sup = 4 if bpw % 4 == 0 else (2 if bpw % 2 == 0 else 1)
    xd = nc.dram_tensor(
        "xd", [nblk // sup, 128, sup, KCH, C + 1], BF16, kind="ExternalInput"
    )
    if XT_MODE == "dma":
        xt = nc.dram_tensor(
            "xt", [nblk // sup, 128, sup, 2, BLK], XT_DT, kind="ExternalInput"
        )
    gl = nc.dram_tensor("gl", [128, nwin * bpw * KCH], FP32, kind="ExternalInput")
    w1 = nc.dram_tensor("w1", [128, 2 * 128], W1_DT, kind="ExternalInput")
    b1v = nc.dram_tensor("b1v", [128, 1], FP32, kind="ExternalInput")
    w2 = nc.dram_tensor("w2", [128, 1], BF16, kind="ExternalInput")
    pooled = nc.dram_tensor("pooled", [nwin * gw, C], FP32, kind="ExternalOutput")

    with tile.TileContext(nc) as tc, ExitStack() as ctx:
        consts = ctx.enter_context(tc.tile_pool(name="consts", bufs=1))
        xp = ctx.enter_context(tc.tile_pool(name="xp", bufs=3))
        xtp = ctx.enter_context(tc.tile_pool(name="xtp", bufs=3))
        thp = ctx.enter_context(tc.tile_pool(name="thp", bufs=2))
        sp = ctx.enter_context(tc.tile_pool(name="sp", bufs=4))
        oep = ctx.enter_context(tc.tile_pool(name="oep", bufs=2))
        outp = ctx.enter_context(tc.tile_pool(name="outp", bufs=2))
        ps_h = ctx.enter_context(tc.tile_pool(name="ps_h", bufs=2, space="PSUM"))
        ps_s = ctx.enter_context(tc.tile_pool(name="ps_s", bufs=2, space="PSUM"))
        ps_acc = ctx.enter_context(tc.tile_pool(name="ps_acc", bufs=2, space="PSUM"))
        if XT_MODE == "pe":
            ps_xt = ctx.enter_context(tc.tile_pool(name="ps_xt", bufs=2, space="PSUM"))

        w1_sb = consts.tile([128, 2, 128], W1_DT)
        nc.sync.dma_start(out=w1_sb[:], in_=w1[:].rearrange("p (cb j) -> p cb j", cb=2))
        b1_sb = consts.tile([128, 1], FP32)
        nc.sync.dma_start(out=b1_sb[:], in_=b1v[:])
        w2_sb = consts.tile([128, 1], BF16)
        nc.sync.dma_start(out=w2_sb[:], in_=w2[:])
        iota_i = consts.tile([128, gw], mybir.dt.int32)
        nc.gpsimd.iota(iota_i[:], pattern=[[1, gw]], base=0, channel_multiplier=0)
        iota_g = consts.tile([128, gw], FP32)
        nc.vector.tensor_copy(out=iota_g[:], in_=iota_i[:])
        if XT_MODE == "pe":
            ident_b = consts.tile([128, 128], BF16)
            make_identity(nc, ident_b[:])
        gl_all = consts.tile([128, nwin, bpw * KCH], FP32)
        nc.sync.dma_start(
            out=gl_all[:], in_=gl[:].rearrange("p (w k) -> p w k", w=nwin)
        )

        # Software pipeline: emit stage A (load, MLP, scores, exp) LOOKAHEAD
        # blocks ahead of stage B (one-hot build + pooling matmuls), so the
        # in-order PE stream always has independent MLP work queued while a
        # block's score chain completes.
        state = {}

        def stage_a(w, b):
            blk = (w % nwin) * bpw + b
            if blk % sup == 0 or ("xsup", w) not in state or state[("xsup", w)][0] != blk // sup:
                xsup = xp.tile([128, sup, KCH, C + 1], BF16, name="xsup", tag="xsup")
                nc.sync.dma_start(out=xsup[:], in_=xd[blk // sup])
                if XT_MODE == "dma":
                    xtsup = xtp.tile([128, sup, 2, BLK], XT_DT, name="xtsup", tag="xtsup")
                    nc.scalar.dma_start(out=xtsup[:], in_=xt[blk // sup])
                else:
                    xtsup = None
                state[("xsup", w)] = (blk // sup, xsup, xtsup)
            _, xsup, xtsup = state[("xsup", w)]
            xb = xsup[:, blk % sup]

            if XT_MODE == "dma":
                xtb = xtsup[:, blk % sup]
            else:
                xt_ps = ps_xt.tile([128, 2, BLK], BF16)  # noqa: used in pe mode
                for cb in range(2):
                    for k in range(KCH):
                        nc.tensor.matmul(
                            out=xt_ps[:, cb, 128 * k : 128 * (k + 1)],
                            lhsT=xb[:, k, 128 * cb : 128 * (cb + 1)],
                            rhs=ident_b[:],
                        )
                xtb = xtp.tile([128, 2, BLK], BF16)
                nc.vector.tensor_copy(out=xtb[:, 0, :], in_=xt_ps[:, 0, :])
                nc.scalar.copy(out=xtb[:, 1, :], in_=xt_ps[:, 1, :])

            h_ps = ps_h.tile([128, BLK], FP32)
            for cb in range(2):
                nc.tensor.matmul(
                    out=h_ps[:],
                    lhsT=w1_sb[:, cb, :],
                    rhs=xtb[:, cb, :],
                    start=(cb == 0),
                    stop=(cb == 1),
                )
            th = thp.tile([128, BLK], BF16)
            nc.scalar.activation(
                out=th[:],
                in_=h_ps[:],
                func=mybir.ActivationFunctionType.Tanh,
                bias=b1_sb[:],
                scale=1.0 / (W1_PRESCALE * X_PRESCALE),
            )

            st_ps = ps_s.tile([128, KCH], FP32)
            for k in range(KCH):
                nc.tensor.matmul(
                    out=st_ps[:, k : k + 1],
                    lhsT=th[:, 128 * k : 128 * (k + 1)],
                    rhs=w2_sb[:],
                )
            st = sp.tile([128, KCH], FP32, tag="st")
            nc.scalar.activation(
                out=st[:], in_=st_ps[:], func=mybir.ActivationFunctionType.Exp
            )
            return w, b, xb, st

        def stage_b(w, b, xb, st):
            first = b == 0
            last = b == bpw - 1
            if first:
                # [gw, 0:256] = pooled numerator, [gw, 256] = denominator.
                # One PSUM region accumulated across the window's blocks.
                state[("acc", w)] = ps_acc.tile([gw, C + 1], FP32, name="acc", tag="acc")
            acc = state[("acc", w)]
            gl_sb = gl_all[:, w % nwin]

            oe = oep.tile([128, KCH, gw], BF16)
            for k in range(KCH):
                nc.vector.tensor_scalar(
                    out=oe[:, k, :],
                    in0=iota_g[:],
                    scalar1=gl_sb[:, KCH * b + k : KCH * b + k + 1],
                    scalar2=st[:, k : k + 1],
                    op0=mybir.AluOpType.is_equal,
                    op1=mybir.AluOpType.mult,
                )

            for k in range(KCH):
                nc.tensor.matmul(
                    out=acc[:],
                    lhsT=oe[:, k, :],
                    rhs=xb[:, k, :],
                    start=(first and k == 0),
                    stop=(last and k == KCH - 1),
                    skip_group_check=True,
                )

            if last:
                d_sb = outp.tile([gw, 1], FP32, tag="d_sb")
                nc.vector.tensor_scalar_add(d_sb[:], acc[:, C : C + 1], 1e-16)
                nc.vector.reciprocal(out=d_sb[:], in_=d_sb[:])
                out_sb = outp.tile([gw, C], FP32, tag="out_sb")
                nc.vector.tensor_scalar_mul(out_sb[:], acc[:, 0:C], d_sb[:])
                # SWDGE (gpsimd) queue: the output DMA waits on the finalize
                # DVE ops; on the sync/SP HWDGE FIFO that wait would stall all
                # input loads queued behind it and starve the DMA engines.
                nc.gpsimd.dma_start(
                    out=pooled[(w % nwin) * gw : (w % nwin + 1) * gw, :],
                    in_=out_sb[:],
                )

        LOOKAHEAD = 5
        blocks = [(w, b) for w in range(repeat * nwin) for b in range(bpw)]
        pending = []
        for (w, b) in blocks:
            pending.append(stage_a(w, b))
            if len(pending) > LOOKAHEAD:
                stage_b(*pending.pop(0))
        for args in pending:
            stage_b(*args)

    nc.compile()
    return nc


def _shard_inputs(x, batch, W1, b1, W2, nw):
    n_graphs = N_CORES * NWIN * GW
    bpw = nw // BLK
    kj = bpw * KCH
    xt_np = ml_dtypes.bfloat16 if XT_DT == BF16 else mybir.dt.np(XT_DT)
    w1_np = ml_dtypes.bfloat16 if W1_DT == BF16 else mybir.dt.np(W1_DT)
    x = np.asarray(x, dtype=np.float32).astype(ml_dtypes.bfloat16)
    batch = np.asarray(batch)

    wstarts = np.searchsorted(batch, np.arange(0, n_graphs + 1, GW))
    W1 = np.asarray(W1, dtype=np.float32)
    w1_host = np.empty((128, 256), dtype=np.float32)
    for cb in range(2):
        w1_host[:, cb * 128 : (cb + 1) * 128] = W1[cb * 128 : (cb + 1) * 128, :]
    w1_host = (w1_host * W1_PRESCALE).astype(w1_np)
    b1_host = np.asarray(b1, dtype=np.float32).reshape(128, 1)
    w2_host = np.asarray(W2, dtype=np.float32).astype(ml_dtypes.bfloat16).reshape(128, 1)

    in_maps = []
    for c in range(N_CORES):
        sup = 4 if bpw % 4 == 0 else (2 if bpw % 2 == 0 else 1)
        xd = np.zeros((NWIN * bpw, 128, KCH, C + 1), dtype=ml_dtypes.bfloat16)
        xtb = (
            np.zeros((NWIN * bpw, 128, 2, BLK), dtype=xt_np)
            if XT_MODE == "dma"
            else None
        )
        glh = np.full((NWIN, 128, kj), -1.0, dtype=np.float32)  # [w][p][k]
        for wl in range(NWIN):
            wg = c * NWIN + wl
            lo, hi = int(wstarts[wg]), int(wstarts[wg + 1])
            cnt = hi - lo
            assert cnt <= nw, f"window {wg} has {cnt} nodes > NW={nw}"
            xpad = np.zeros((nw, C + 1), dtype=ml_dtypes.bfloat16)
            xpad[:cnt, :C] = x[lo:hi]
            xpad[:, C] = 1.0
            xd[wl * bpw : (wl + 1) * bpw] = xpad.reshape(bpw, KCH, 128, C + 1).transpose(
                0, 2, 1, 3
            )
            if xtb is not None:
                xs = np.ascontiguousarray(
                    xpad[:, :C].astype(np.float32).T * X_PRESCALE
                ).astype(xt_np)  # [C, nw]
                xtb[wl * bpw : (wl + 1) * bpw] = (
                    xs.reshape(2, 128, bpw, BLK).transpose(2, 1, 0, 3)
                )
            glpad = np.full((nw,), -1.0, dtype=np.float32)
            glpad[:cnt] = (batch[lo:hi] - wg * GW).astype(np.float32)
            glh[wl] = glpad.reshape(bpw, KCH, 128).transpose(2, 0, 1).reshape(128, kj)
        glh_flat = glh.transpose(1, 0, 2).reshape(128, NWIN * kj)
        nsup = NWIN * bpw // sup
        xd_s = xd.reshape(nsup, sup, 128, KCH, C + 1).transpose(0, 2, 1, 3, 4)
        m = {"xd": np.ascontiguousarray(xd_s), "gl": glh_flat, "w1": w1_host,
             "b1v": b1_host, "w2": w2_host}
        if xtb is not None:
            xt_s = xtb.reshape(nsup, sup, 128, 2, BLK).transpose(0, 2, 1, 3, 4)
            m["xt"] = np.ascontiguousarray(xt_s)
        in_maps.append(m)
    return in_maps


def kernel(x, batch, W1, b1, W2, b2):
    x = np.asarray(x)
    batch = np.asarray(batch)
    n_graphs = N_CORES * NWIN * GW
    assert x.shape[1] == C and batch.shape[0] == x.shape[0]

    # padded nodes per window, from the actual data
    wstarts = np.searchsorted(batch, np.arange(0, n_graphs + 1, GW))
    max_win = int(np.diff(wstarts).max())
    nw = max(BLK, -(-max_win // BLK) * BLK)

    key = (NWIN, nw, GW)
    if key not in _NC_CACHE:
        _NC_CACHE[key] = _build_nc(*key)
    nc = _NC_CACHE[key]

    in_maps = _shard_inputs(x, batch, W1, b1, W2, nw)
    res = bass_utils.run_bass_kernel_spmd(
        nc,
        in_maps,
        core_ids=list(range(N_CORES)),
    )
    out = np.concatenate(
        [res.results[c]["pooled"] for c in range(N_CORES)], axis=0
    ).astype(np.float32)
    return out


# revision 3
# speedup vs baseline: 2.1128x; 2.1128x over previous
"""AttentionPool (segment softmax + weighted segment sum) on 8 TRN2 cores, v2.

kernel(x, batch, W1, b1, W2, b2) -> pooled [4096, 256] f32

Differences vs the v1 baseline (which was PE/ACT-bound on 8 PE-transposes +
PSUM->SBUF copies per 512-node block):
  - x is shipped in BOTH layouts from the host: node-on-partition blocks
    (`xd`, with a 257th ones-column for the softmax denominator) for the
    pooling matmul, and channel-on-partition blocks (`xt`) for the MLP.
    No on-device transposes, no PSUM->SBUF copies (XT_MODE="pe" restores the
    on-device transpose variant for comparison).
  - Pooling uses the one-hot weights oe as the *stationary* operand
    (32-column LDWEIGHTS) and x as the moving operand; output accumulates
    as [gw, 257] (graph-on-partition) in PSUM across the window's blocks,
    col 256 = denominator. Finalize is 3 DVE ops + DMA per window; the v1
    final PE transposes are gone.

Per 512-node block the PE does: 2 MLP matmuls (512 mov each), 4 score
matmuls (1 mov), 4 pooling matmuls (257 mov) -> ~2.06k moving columns.
"""

from contextlib import ExitStack

import numpy as np
import ml_dtypes

import concourse.bass as bass
import concourse.mybir as mybir
import concourse.tile as tile
from concourse import bacc, bass_utils
from concourse.masks import make_identity

FP32 = mybir.dt.float32
BF16 = mybir.dt.bfloat16
FP8 = mybir.dt.float8e4
C = 256
BLK = 512
KCH = BLK // 128
N_CORES = 8
GW = 32  # graphs per window
NWIN = 16  # windows per core -> 512 graphs per core

XT_MODE = "dma"  # "dma": host ships x^T; "pe": transpose on tensor engine
XT_DT = FP8  # dtype of the score-path x^T (BF16 or FP8 = float8e3, 4 mantissa bits)
# Pre-scales shift x and W1 into e3m4's normal range (min normal 0.25, max
# 15.5) so PE subnormal flushing costs nothing; descaled in tanh's scale.
X_PRESCALE = 2.8
W1_PRESCALE = 1.0
W1_DT = BF16  # stationary may stay bf16 while the moving operand is fp8

_NC_CACHE = {}


def _build_nc(nwin, nw, gw, repeat=1):
    # repeat > 1 re-runs the whole computation (same inputs/outputs) inside
    # one NEFF; used only for overhead-cancelling timing measurements.
    assert nw % BLK == 0 and gw <= 128
    bpw = nw // BLK
    nblk = nwin * bpw

    nc = bacc.Bacc(None, target_bir_lowering=False)

    sup = 4 if bpw % 4 == 0 else (2 if bpw % 2 == 0 else 1)
    xd = nc.dram_tensor(
        "xd", [nblk // sup, 128, sup, KCH, C + 1], BF16, kind="ExternalInput"
    )
    if XT_MODE == "dma":
        xt = nc.dram_tensor(
            "xt", [nblk // sup, 128, sup, 2, BLK], XT_DT, kind="ExternalInput"
        )
    gl = nc.dram_tensor("gl", [128, nwin * bpw * KCH], FP32, kind="ExternalInput")
    w1 = nc.dram_tensor("w1", [128, 2 * 128], W1_DT, kind="ExternalInput")
    b1v = nc.dram_tensor("b1v", [128, 1], FP32, kind="ExternalInput")
    w2 = nc.dram_tensor("w2", [128, 1], BF16, kind="ExternalInput")
    pooled = nc.dram_tensor("pooled", [nwin * gw, C], FP32, kind="ExternalOutput")

    with tile.TileContext(nc) as tc, ExitStack() as ctx:
        consts = ctx.enter_context(tc.tile_pool(name="consts", bufs=1))
        xp = ctx.enter_context(tc.tile_pool(name="xp", bufs=3))
        xtp = ctx.enter_context(tc.tile_pool(name="xtp", bufs=3))
        thp = ctx.enter_context(tc.tile_pool(name="thp", bufs=2))
        sp = ctx.enter_context(tc.tile_pool(name="sp", bufs=4))
        oep = ctx.enter_context(tc.tile_pool(name="oep", bufs=2))
        outp = ctx.enter_context(tc.tile_pool(name="outp", bufs=2))
        ps_h = ctx.enter_context(tc.tile_pool(name="ps_h", bufs=2, space="PSUM"))
        ps_s = ctx.enter_context(tc.tile_pool(name="ps_s", bufs=2, space="PSUM"))
        ps_acc = ctx.enter_context(tc.tile_pool(name="ps_acc", bufs=2, space="PSUM"))
        if XT_MODE == "pe":
            ps_xt = ctx.enter_context(tc.tile_pool(name="ps_xt", bufs=2, space="PSUM"))

        w1_sb = consts.tile([128, 2, 128], W1_DT)
        nc.sync.dma_start(out=w1_sb[:], in_=w1[:].rearrange("p (cb j) -> p cb j", cb=2))
        b1_sb = consts.tile([128, 1], FP32)
        nc.sync.dma_start(out=b1_sb[:], in_=b1v[:])
        w2_sb = consts.tile([128, 1], BF16)
        nc.sync.dma_start(out=w2_sb[:], in_=w2[:])
        iota_i = consts.tile([128, gw], mybir.dt.int32)
        nc.gpsimd.iota(iota_i[:], pattern=[[1, gw]], base=0, channel_multiplier=0)
        iota_g = consts.tile([128, gw], FP32)
        nc.vector.tensor_copy(out=iota_g[:], in_=iota_i[:])
        if XT_MODE == "pe":
            ident_b = consts.tile([128, 128], BF16)
            make_identity(nc, ident_b[:])
        gl_all = consts.tile([128, nwin, bpw * KCH], FP32)
        nc.sync.dma_start(
            out=gl_all[:], in_=gl[:].rearrange("p (w k) -> p w k", w=nwin)
        )

        SGRP = 4 if bpw % 4 == 0 else (2 if bpw % 2 == 0 else 1)
        # Software pipeline: emit stage A (load, MLP, scores, exp) LOOKAHEAD
        # blocks ahead of stage B (one-hot build + pooling matmuls), so the
        # in-order PE stream always has independent MLP work queued while a
        # block's score chain completes.
        state = {}

        def stage_a(w, b):
            blk = (w % nwin) * bpw + b
            if blk % sup == 0 or ("xsup", w) not in state or state[("xsup", w)][0] != blk // sup:
                xsup = xp.tile([128, sup, KCH, C + 1], BF16, name="xsup", tag="xsup")
                nc.sync.dma_start(out=xsup[:], in_=xd[blk // sup])
                if XT_MODE == "dma":
                    xtsup = xtp.tile([128, sup, 2, BLK], XT_DT, name="xtsup", tag="xtsup")
                    nc.scalar.dma_start(out=xtsup[:], in_=xt[blk // sup])
                else:
                    xtsup = None
                state[("xsup", w)] = (blk // sup, xsup, xtsup)
            _, xsup, xtsup = state[("xsup", w)]
            xb = xsup[:, blk % sup]

            if XT_MODE == "dma":
                xtb = xtsup[:, blk % sup]
            else:
                xt_ps = ps_xt.tile([128, 2, BLK], BF16)  # noqa: used in pe mode
                for cb in range(2):
                    for k in range(KCH):
                        nc.tensor.matmul(
                            out=xt_ps[:, cb, 128 * k : 128 * (k + 1)],
                            lhsT=xb[:, k, 128 * cb : 128 * (cb + 1)],
                            rhs=ident_b[:],
                        )
                xtb = xtp.tile([128, 2, BLK], BF16)
                nc.vector.tensor_copy(out=xtb[:, 0, :], in_=xt_ps[:, 0, :])
                nc.scalar.copy(out=xtb[:, 1, :], in_=xt_ps[:, 1, :])

            h_ps = ps_h.tile([128, BLK], FP32)
            for cb in range(2):
                nc.tensor.matmul(
                    out=h_ps[:],
                    lhsT=w1_sb[:, cb, :],
                    rhs=xtb[:, cb, :],
                    start=(cb == 0),
                    stop=(cb == 1),
                )
            th = thp.tile([128, BLK], BF16)
            nc.scalar.activation(
                out=th[:],
                in_=h_ps[:],
                func=mybir.ActivationFunctionType.Tanh,
                bias=b1_sb[:],
                scale=1.0 / (W1_PRESCALE * X_PRESCALE),
            )

            # scores for SGRP consecutive blocks share one PSUM tile and one
            # exp activation (exp on [128, KCH] alone is overhead-dominated).
            g, gi = b // SGRP, b % SGRP
            if gi == 0:
                state[("st_ps", w, g)] = ps_s.tile(
                    [128, SGRP, KCH], FP32, name="st_ps", tag="st_ps"
                )
            st_ps = state[("st_ps", w, g)]
            for k in range(KCH):
                nc.tensor.matmul(
                    out=st_ps[:, gi, k : k + 1],
                    lhsT=th[:, 128 * k : 128 * (k + 1)],
                    rhs=w2_sb[:],
                )
            if gi == SGRP - 1:
                st4 = sp.tile([128, SGRP, KCH], FP32, tag="st")
                nc.scalar.activation(
                    out=st4[:], in_=st_ps[:], func=mybir.ActivationFunctionType.Exp
                )
                state[("st", w, g)] = st4
            return w, b, xb

        def stage_b(w, b, xb):
            st = state[("st", w, b // SGRP)][:, b % SGRP]
            first = b == 0
            last = b == bpw - 1
            if first:
                # [gw, 0:256] = pooled numerator, [gw, 256] = denominator.
                # One PSUM region accumulated across the window's blocks.
                state[("acc", w)] = ps_acc.tile([gw, C + 1], FP32, name="acc", tag="acc")
            acc = state[("acc", w)]
            gl_sb = gl_all[:, w % nwin]

            oe = oep.tile([128, KCH, gw], BF16)
            for k in range(KCH):
                nc.vector.tensor_scalar(
                    out=oe[:, k, :],
                    in0=iota_g[:],
                    scalar1=gl_sb[:, KCH * b + k : KCH * b + k + 1],
                    scalar2=st[:, k : k + 1],
                    op0=mybir.AluOpType.is_equal,
                    op1=mybir.AluOpType.mult,
                )

            for k in range(KCH):
                nc.tensor.matmul(
                    out=acc[:],
                    lhsT=oe[:, k, :],
                    rhs=xb[:, k, :],
                    start=(first and k == 0),
                    stop=(last and k == KCH - 1),
                    skip_group_check=True,
                )

            if last:
                d_sb = outp.tile([gw, 1], FP32, tag="d_sb")
                nc.vector.tensor_scalar_add(d_sb[:], acc[:, C : C + 1], 1e-16)
                nc.vector.reciprocal(out=d_sb[:], in_=d_sb[:])
                out_sb = outp.tile([gw, C], FP32, tag="out_sb")
                nc.vector.tensor_scalar_mul(out_sb[:], acc[:, 0:C], d_sb[:])
                # SWDGE (gpsimd) queue: the output DMA waits on the finalize
                # DVE ops; on the sync/SP HWDGE FIFO that wait would stall all
                # input loads queued behind it and starve the DMA engines.
                nc.gpsimd.dma_start(
                    out=pooled[(w % nwin) * gw : (w % nwin + 1) * gw, :],
                    in_=out_sb[:],
                )

        LOOKAHEAD = 5
        assert LOOKAHEAD >= SGRP
        blocks = [(w, b) for w in range(repeat * nwin) for b in range(bpw)]
        pending = []
        for (w, b) in blocks:
            pending.append(stage_a(w, b))
            if len(pending) > LOOKAHEAD:
                stage_b(*pending.pop(0))
        for args in pending:
            stage_b(*args)

    nc.compile()
    return nc


def _shard_inputs(x, batch, W1, b1, W2, nw):
    n_graphs = N_CORES * NWIN * GW
    bpw = nw // BLK
    kj = bpw * KCH
    xt_np = ml_dtypes.bfloat16 if XT_DT == BF16 else mybir.dt.np(XT_DT)
    w1_np = ml_dtypes.bfloat16 if W1_DT == BF16 else mybir.dt.np(W1_DT)
    x = np.asarray(x, dtype=np.float32).astype(ml_dtypes.bfloat16)
    batch = np.asarray(batch)

    wstarts = np.searchsorted(batch, np.arange(0, n_graphs + 1, GW))
    W1 = np.asarray(W1, dtype=np.float32)
    w1_host = np.empty((128, 256), dtype=np.float32)
    for cb in range(2):
        w1_host[:, cb * 128 : (cb + 1) * 128] = W1[cb * 128 : (cb + 1) * 128, :]
    w1_host = (w1_host * W1_PRESCALE).astype(w1_np)
    b1_host = np.asarray(b1, dtype=np.float32).reshape(128, 1)
    w2_host = np.asarray(W2, dtype=np.float32).astype(ml_dtypes.bfloat16).reshape(128, 1)

    in_maps = []
    for c in range(N_CORES):
        sup = 4 if bpw % 4 == 0 else (2 if bpw % 2 == 0 else 1)
        xd = np.zeros((NWIN * bpw, 128, KCH, C + 1), dtype=ml_dtypes.bfloat16)
        xtb = (
            np.zeros((NWIN * bpw, 128, 2, BLK), dtype=xt_np)
            if XT_MODE == "dma"
            else None
        )
        glh = np.full((NWIN, 128, kj), -1.0, dtype=np.float32)  # [w][p][k]
        for wl in range(NWIN):
            wg = c * NWIN + wl
            lo, hi = int(wstarts[wg]), int(wstarts[wg + 1])
            cnt = hi - lo
            assert cnt <= nw, f"window {wg} has {cnt} nodes > NW={nw}"
            xpad = np.zeros((nw, C + 1), dtype=ml_dtypes.bfloat16)
            xpad[:cnt, :C] = x[lo:hi]
            xpad[:, C] = 1.0
            xd[wl * bpw : (wl + 1) * bpw] = xpad.reshape(bpw, KCH, 128, C + 1).transpose(
                0, 2, 1, 3
            )
            if xtb is not None:
                xs = np.ascontiguousarray(
                    xpad[:, :C].astype(np.float32).T * X_PRESCALE
                ).astype(xt_np)  # [C, nw]
                xtb[wl * bpw : (wl + 1) * bpw] = (
                    xs.reshape(2, 128, bpw, BLK).transpose(2, 1, 0, 3)
                )
            glpad = np.full((nw,), -1.0, dtype=np.float32)
            glpad[:cnt] = (batch[lo:hi] - wg * GW).astype(np.float32)
            glh[wl] = glpad.reshape(bpw, KCH, 128).transpose(2, 0, 1).reshape(128, kj)
        glh_flat = glh.transpose(1, 0, 2).reshape(128, NWIN * kj)
        nsup = NWIN * bpw // sup
        xd_s = xd.reshape(nsup, sup, 128, KCH, C + 1).transpose(0, 2, 1, 3, 4)
        m = {"xd": np.ascontiguousarray(xd_s), "gl": glh_flat, "w1": w1_host,
             "b1v": b1_host, "w2": w2_host}
        if xtb is not None:
            xt_s = xtb.reshape(nsup, sup, 128, 2, BLK).transpose(0, 2, 1, 3, 4)
            m["xt"] = np.ascontiguousarray(xt_s)
        in_maps.append(m)
    return in_maps


def kernel(x, batch, W1, b1, W2, b2):
    x = np.asarray(x)
    batch = np.asarray(batch)
    n_graphs = N_CORES * NWIN * GW
    assert x.shape[1] == C and batch.shape[0] == x.shape[0]

    # padded nodes per window, from the actual data
    wstarts = np.searchsorted(batch, np.arange(0, n_graphs + 1, GW))
    max_win = int(np.diff(wstarts).max())
    nw = max(BLK, -(-max_win // BLK) * BLK)

    key = (NWIN, nw, GW)
    if key not in _NC_CACHE:
        _NC_CACHE[key] = _build_nc(*key)
    nc = _NC_CACHE[key]

    in_maps = _shard_inputs(x, batch, W1, b1, W2, nw)
    res = bass_utils.run_bass_kernel_spmd(
        nc,
        in_maps,
        core_ids=list(range(N_CORES)),
    )
    out = np.concatenate(
        [res.results[c]["pooled"] for c in range(N_CORES)], axis=0
    ).astype(np.float32)
    return out


# revision 4
# speedup vs baseline: 2.8328x; 1.3408x over previous
"""AttentionPool (segment softmax + weighted segment sum) on 8 TRN2 cores, v2.

kernel(x, batch, W1, b1, W2, b2) -> pooled [4096, 256] f32

Differences vs the v1 baseline (which was PE/ACT-bound on 8 PE-transposes +
PSUM->SBUF copies per 512-node block):
  - x is shipped in BOTH layouts from the host: node-on-partition blocks
    (`xd`, with a 257th ones-column for the softmax denominator) for the
    pooling matmul, and channel-on-partition blocks (`xt`) for the MLP.
    No on-device transposes, no PSUM->SBUF copies (XT_MODE="pe" restores the
    on-device transpose variant for comparison).
  - Pooling uses the one-hot weights oe as the *stationary* operand
    (32-column LDWEIGHTS) and x as the moving operand; output accumulates
    as [gw, 257] (graph-on-partition) in PSUM across the window's blocks,
    col 256 = denominator. Finalize is 3 DVE ops + DMA per window; the v1
    final PE transposes are gone.

Per 512-node block the PE does: 2 MLP matmuls (512 mov each), 4 score
matmuls (1 mov), 4 pooling matmuls (257 mov) -> ~2.06k moving columns.
"""

from contextlib import ExitStack

import numpy as np
import ml_dtypes

import concourse.bass as bass
import concourse.mybir as mybir
import concourse.tile as tile
from concourse import bacc, bass_utils
from concourse.masks import make_identity

FP32 = mybir.dt.float32
BF16 = mybir.dt.bfloat16
FP8 = mybir.dt.float8e4
C = 256
BLK = 512
KCH = BLK // 128
N_CORES = 8
GW = 32  # graphs per window
NWIN = 16  # windows per core -> 512 graphs per core

XT_MODE = "dma"  # "dma": host ships x^T; "pe": transpose on tensor engine
XT_DT = FP8  # dtype of the score-path x^T (BF16 or FP8 = float8e3, 4 mantissa bits)
# Pre-scales shift x and W1 into e3m4's normal range (min normal 0.25, max
# 15.5) so PE subnormal flushing costs nothing; descaled in tanh's scale.
X_PRESCALE = 2.8
W1_PRESCALE = 1.0
W1_DT = BF16  # stationary may stay bf16 while the moving operand is fp8

_NC_CACHE = {}


def _build_nc(nwin, nw, gw, repeat=1):
    # repeat > 1 re-runs the whole computation (same inputs/outputs) inside
    # one NEFF; used only for overhead-cancelling timing measurements.
    assert nw % BLK == 0 and gw <= 128
    bpw = nw // BLK
    nblk = nwin * bpw

    nc = bacc.Bacc(None, target_bir_lowering=False)

    sup = 4 if bpw % 4 == 0 else (2 if bpw % 2 == 0 else 1)
    xd = nc.dram_tensor(
        "xd", [nblk // sup, 128, sup, KCH, C + 1], BF16, kind="ExternalInput"
    )
    if XT_MODE == "dma":
        xt = nc.dram_tensor(
            "xt", [nblk // sup, 128, sup, 2, BLK], XT_DT, kind="ExternalInput"
        )
    gl = nc.dram_tensor("gl", [128, nwin * bpw * KCH], FP32, kind="ExternalInput")
    w1 = nc.dram_tensor("w1", [128, 2 * 128], W1_DT, kind="ExternalInput")
    b1v = nc.dram_tensor("b1v", [128, 1], FP32, kind="ExternalInput")
    w2 = nc.dram_tensor("w2", [128, 1], BF16, kind="ExternalInput")
    pooled = nc.dram_tensor("pooled", [nwin * gw, C], FP32, kind="ExternalOutput")

    with tile.TileContext(nc) as tc, ExitStack() as ctx:
        consts = ctx.enter_context(tc.tile_pool(name="consts", bufs=1))
        xp = ctx.enter_context(tc.tile_pool(name="xp", bufs=4))
        xtp = ctx.enter_context(tc.tile_pool(name="xtp", bufs=4))
        thp = ctx.enter_context(tc.tile_pool(name="thp", bufs=2))
        sp = ctx.enter_context(tc.tile_pool(name="sp", bufs=4))
        oep = ctx.enter_context(tc.tile_pool(name="oep", bufs=2))
        outp = ctx.enter_context(tc.tile_pool(name="outp", bufs=2))
        ps_h = ctx.enter_context(tc.tile_pool(name="ps_h", bufs=2, space="PSUM"))
        ps_s = ctx.enter_context(tc.tile_pool(name="ps_s", bufs=2, space="PSUM"))
        ps_acc = ctx.enter_context(tc.tile_pool(name="ps_acc", bufs=2, space="PSUM"))
        if XT_MODE == "pe":
            ps_xt = ctx.enter_context(tc.tile_pool(name="ps_xt", bufs=2, space="PSUM"))

        w1_sb = consts.tile([128, 2, 128], W1_DT)
        nc.sync.dma_start(out=w1_sb[:], in_=w1[:].rearrange("p (cb j) -> p cb j", cb=2))
        b1_sb = consts.tile([128, 1], FP32)
        nc.sync.dma_start(out=b1_sb[:], in_=b1v[:])
        w2_sb = consts.tile([128, 1], BF16)
        nc.sync.dma_start(out=w2_sb[:], in_=w2[:])
        iota_i = consts.tile([128, gw], mybir.dt.int32)
        nc.gpsimd.iota(iota_i[:], pattern=[[1, gw]], base=0, channel_multiplier=0)
        iota_g = consts.tile([128, gw], FP32)
        nc.vector.tensor_copy(out=iota_g[:], in_=iota_i[:])
        if XT_MODE == "pe":
            ident_b = consts.tile([128, 128], BF16)
            make_identity(nc, ident_b[:])
        gl_all = consts.tile([128, nwin, bpw * KCH], FP32)
        # scalar (ACT) HWDGE ring: keeps the 512 KB gl load off the sync ring
        # so the first xd supers start streaming immediately.
        nc.scalar.dma_start(
            out=gl_all[:], in_=gl[:].rearrange("p (w k) -> p w k", w=nwin)
        )

        SGRP = 4 if bpw % 4 == 0 else (2 if bpw % 2 == 0 else 1)
        # Software pipeline: emit stage A (load, MLP, scores, exp) LOOKAHEAD
        # blocks ahead of stage B (one-hot build + pooling matmuls), so the
        # in-order PE stream always has independent MLP work queued while a
        # block's score chain completes.
        state = {}

        def stage_a(w, b):
            blk = (w % nwin) * bpw + b
            if blk % sup == 0 or ("xsup", w) not in state or state[("xsup", w)][0] != blk // sup:
                xsup = xp.tile([128, sup, KCH, C + 1], BF16, name="xsup", tag="xsup")
                nc.sync.dma_start(out=xsup[:], in_=xd[blk // sup])
                if XT_MODE == "dma":
                    xtsup = xtp.tile([128, sup, 2, BLK], XT_DT, name="xtsup", tag="xtsup")
                    nc.scalar.dma_start(out=xtsup[:], in_=xt[blk // sup])
                else:
                    xtsup = None
                state[("xsup", w)] = (blk // sup, xsup, xtsup)
            _, xsup, xtsup = state[("xsup", w)]
            xb = xsup[:, blk % sup]

            if XT_MODE == "dma":
                xtb = xtsup[:, blk % sup]
            else:
                xt_ps = ps_xt.tile([128, 2, BLK], BF16)  # noqa: used in pe mode
                for cb in range(2):
                    for k in range(KCH):
                        nc.tensor.matmul(
                            out=xt_ps[:, cb, 128 * k : 128 * (k + 1)],
                            lhsT=xb[:, k, 128 * cb : 128 * (cb + 1)],
                            rhs=ident_b[:],
                        )
                xtb = xtp.tile([128, 2, BLK], BF16)
                nc.vector.tensor_copy(out=xtb[:, 0, :], in_=xt_ps[:, 0, :])
                nc.scalar.copy(out=xtb[:, 1, :], in_=xt_ps[:, 1, :])

            h_ps = ps_h.tile([128, BLK], FP32)
            for cb in range(2):
                nc.tensor.matmul(
                    out=h_ps[:],
                    lhsT=w1_sb[:, cb, :],
                    rhs=xtb[:, cb, :],
                    start=(cb == 0),
                    stop=(cb == 1),
                )
            th = thp.tile([128, BLK], BF16)
            nc.scalar.activation(
                out=th[:],
                in_=h_ps[:],
                func=mybir.ActivationFunctionType.Tanh,
                bias=b1_sb[:],
                scale=1.0 / (W1_PRESCALE * X_PRESCALE),
            )

            # scores for SGRP consecutive blocks share one PSUM tile and one
            # exp activation (exp on [128, KCH] alone is overhead-dominated).
            g, gi = b // SGRP, b % SGRP
            if gi == 0:
                state[("st_ps", w, g)] = ps_s.tile(
                    [128, SGRP, KCH], FP32, name="st_ps", tag="st_ps"
                )
            st_ps = state[("st_ps", w, g)]
            for k in range(KCH):
                nc.tensor.matmul(
                    out=st_ps[:, gi, k : k + 1],
                    lhsT=th[:, 128 * k : 128 * (k + 1)],
                    rhs=w2_sb[:],
                )
            if gi == SGRP - 1:
                st4 = sp.tile([128, SGRP, KCH], FP32, tag="st")
                nc.scalar.activation(
                    out=st4[:], in_=st_ps[:], func=mybir.ActivationFunctionType.Exp
                )
                state[("st", w, g)] = st4
            return w, b, xb

        def stage_b(w, b, xb):
            st = state[("st", w, b // SGRP)][:, b % SGRP]
            first = b == 0
            last = b == bpw - 1
            if first:
                # [gw, 0:256] = pooled numerator, [gw, 256] = denominator.
                # One PSUM region accumulated across the window's blocks.
                state[("acc", w)] = ps_acc.tile([gw, C + 1], FP32, name="acc", tag="acc")
            acc = state[("acc", w)]
            gl_sb = gl_all[:, w % nwin]

            oe = oep.tile([128, KCH, gw], BF16)
            for k in range(KCH):
                nc.vector.tensor_scalar(
                    out=oe[:, k, :],
                    in0=iota_g[:],
                    scalar1=gl_sb[:, KCH * b + k : KCH * b + k + 1],
                    scalar2=st[:, k : k + 1],
                    op0=mybir.AluOpType.is_equal,
                    op1=mybir.AluOpType.mult,
                )

            for k in range(KCH):
                nc.tensor.matmul(
                    out=acc[:],
                    lhsT=oe[:, k, :],
                    rhs=xb[:, k, :],
                    start=(first and k == 0),
                    stop=(last and k == KCH - 1),
                    skip_group_check=True,
                )

            if last:
                d_sb = outp.tile([gw, 1], FP32, tag="d_sb")
                nc.vector.tensor_scalar_add(d_sb[:], acc[:, C : C + 1], 1e-16)
                nc.vector.reciprocal(out=d_sb[:], in_=d_sb[:])
                out_sb = outp.tile([gw, C], FP32, tag="out_sb")
                nc.vector.tensor_scalar_mul(out_sb[:], acc[:, 0:C], d_sb[:])
                # SWDGE (gpsimd) queue: the output DMA waits on the finalize
                # DVE ops; on the sync/SP HWDGE FIFO that wait would stall all
                # input loads queued behind it and starve the DMA engines.
                nc.gpsimd.dma_start(
                    out=pooled[(w % nwin) * gw : (w % nwin + 1) * gw, :],
                    in_=out_sb[:],
                )

        LOOKAHEAD = 6
        assert LOOKAHEAD >= SGRP
        blocks = [(w, b) for w in range(repeat * nwin) for b in range(bpw)]
        pending = []
        for (w, b) in blocks:
            pending.append(stage_a(w, b))
            if len(pending) > LOOKAHEAD:
                stage_b(*pending.pop(0))
        for args in pending:
            stage_b(*args)

    nc.compile()
    return nc


def _shard_inputs(x, batch, W1, b1, W2, nw):
    n_graphs = N_CORES * NWIN * GW
    bpw = nw // BLK
    kj = bpw * KCH
    xt_np = ml_dtypes.bfloat16 if XT_DT == BF16 else mybir.dt.np(XT_DT)
    w1_np = ml_dtypes.bfloat16 if W1_DT == BF16 else mybir.dt.np(W1_DT)
    x = np.asarray(x, dtype=np.float32).astype(ml_dtypes.bfloat16)
    batch = np.asarray(batch)

    wstarts = np.searchsorted(batch, np.arange(0, n_graphs + 1, GW))
    W1 = np.asarray(W1, dtype=np.float32)
    w1_host = np.empty((128, 256), dtype=np.float32)
    for cb in range(2):
        w1_host[:, cb * 128 : (cb + 1) * 128] = W1[cb * 128 : (cb + 1) * 128, :]
    w1_host = (w1_host * W1_PRESCALE).astype(w1_np)
    b1_host = np.asarray(b1, dtype=np.float32).reshape(128, 1)
    w2_host = np.asarray(W2, dtype=np.float32).astype(ml_dtypes.bfloat16).reshape(128, 1)

    in_maps = []
    for c in range(N_CORES):
        sup = 4 if bpw % 4 == 0 else (2 if bpw % 2 == 0 else 1)
        xd = np.zeros((NWIN * bpw, 128, KCH, C + 1), dtype=ml_dtypes.bfloat16)
        xtb = (
            np.zeros((NWIN * bpw, 128, 2, BLK), dtype=xt_np)
            if XT_MODE == "dma"
            else None
        )
        glh = np.full((NWIN, 128, kj), -1.0, dtype=np.float32)  # [w][p][k]
        for wl in range(NWIN):
            wg = c * NWIN + wl
            lo, hi = int(wstarts[wg]), int(wstarts[wg + 1])
            cnt = hi - lo
            assert cnt <= nw, f"window {wg} has {cnt} nodes > NW={nw}"
            xpad = np.zeros((nw, C + 1), dtype=ml_dtypes.bfloat16)
            xpad[:cnt, :C] = x[lo:hi]
            xpad[:, C] = 1.0
            xd[wl * bpw : (wl + 1) * bpw] = xpad.reshape(bpw, KCH, 128, C + 1).transpose(
                0, 2, 1, 3
            )
            if xtb is not None:
                xs = np.ascontiguousarray(
                    xpad[:, :C].astype(np.float32).T * X_PRESCALE
                ).astype(xt_np)  # [C, nw]
                xtb[wl * bpw : (wl + 1) * bpw] = (
                    xs.reshape(2, 128, bpw, BLK).transpose(2, 1, 0, 3)
                )
            glpad = np.full((nw,), -1.0, dtype=np.float32)
            glpad[:cnt] = (batch[lo:hi] - wg * GW).astype(np.float32)
            glh[wl] = glpad.reshape(bpw, KCH, 128).transpose(2, 0, 1).reshape(128, kj)
        glh_flat = glh.transpose(1, 0, 2).reshape(128, NWIN * kj)
        nsup = NWIN * bpw // sup
        xd_s = xd.reshape(nsup, sup, 128, KCH, C + 1).transpose(0, 2, 1, 3, 4)
        m = {"xd": np.ascontiguousarray(xd_s), "gl": glh_flat, "w1": w1_host,
             "b1v": b1_host, "w2": w2_host}
        if xtb is not None:
            xt_s = xtb.reshape(nsup, sup, 128, 2, BLK).transpose(0, 2, 1, 3, 4)
            m["xt"] = np.ascontiguousarray(xt_s)
        in_maps.append(m)
    return in_maps


def kernel(x, batch, W1, b1, W2, b2):
    x = np.asarray(x)
    batch = np.asarray(batch)
    n_graphs = N_CORES * NWIN * GW
    assert x.shape[1] == C and batch.shape[0] == x.shape[0]

    # padded nodes per window, from the actual data
    wstarts = np.searchsorted(batch, np.arange(0, n_graphs + 1, GW))
    max_win = int(np.diff(wstarts).max())
    nw = max(BLK, -(-max_win // BLK) * BLK)

    key = (NWIN, nw, GW)
    if key not in _NC_CACHE:
        _NC_CACHE[key] = _build_nc(*key)
    nc = _NC_CACHE[key]

    in_maps = _shard_inputs(x, batch, W1, b1, W2, nw)
    res = bass_utils.run_bass_kernel_spmd(
        nc,
        in_maps,
        core_ids=list(range(N_CORES)),
    )
    out = np.concatenate(
        [res.results[c]["pooled"] for c in range(N_CORES)], axis=0
    ).astype(np.float32)
    return out
